# revision 25
# baseline (speedup 1.0000x reference)
"""BoundaryAttentionModule Trainium2 kernel.

Shapes (hardcoded): b=4, c=256, h=w=64 (HW=4096), boundary 128x128,
mid=64, out_ch=256. 8 cores: core = (batch bi = core//2, key-half kh = core%2).

Math (exact reassociation of the reference):
  bm   = nearest-downsampled boundary map        [b, 4096]
  R    = relu(kw1f outer bm_khalf + beta)        [64, 2048]   (kw1f = key_w1*bn_inv)
  G    = (key_w2^T @ query_w) @ u                [64, 4096]
  E^T  = R^T @ G                                 [2048_k, 4096_j]  (logits tiny, no max sub)
  U    = exp(E^T), s[k] = sum_j U[k, j]
  Vt   = (u^T @ value_w^T)[k_half] / s * 8192    [2048, 256]
  P    = Vt^T @ U                                [256, 4096]  per-core partial (x8192)
host: out[bi] = (gamma/8192) * (P[2bi] + P[2bi+1]) + u[bi]

Energy matmuls in bf16 with K=64 contraction packed as concurrent
partition-half duos (R and G are host/device-duplicated into both
partition halves, so two j-slices of one k-tile run in the PE array
simultaneously).  The output matmul runs in fp8e4 DoubleRow (2 keys per
cell); the key axis is host-permuted pairwise so PSUM partitions
interleave — the permutation only reorders the contracted axis.
"""

import numpy as np

B, C, HW = 4, 256, 4096
KH = HW // 2          # 2048 keys per core
NK = KH // 128        # 16 k tiles
NP = NK // 2          # 8 k-tile pairs
MID = 64
VSCALE = 8192.0       # fp8 scaling of Vt (power of two; host divides gamma)

TRACE = False
TRACE_CORES = None
LAST_RESULTS = None

_BUILT = None


def _build():
    import concourse.bass as bass
    import concourse.tile as tile
    from concourse import bacc, mybir

    f32 = mybir.dt.float32
    bf16 = mybir.dt.bfloat16
    fp8 = mybir.dt.float8e4
    AF = mybir.ActivationFunctionType
    AX = mybir.AxisListType
    ALU = mybir.AluOpType

    nc = bacc.Bacc(
        "TRN2",
        target_bir_lowering=False,
        debug=False,
        enable_asserts=False,
        num_devices=8,
    )

    u_in = nc.dram_tensor("u_in", [C, HW], bf16, kind="ExternalInput").ap()
    uk_in = nc.dram_tensor("uk_in", [C, KH], bf16, kind="ExternalInput").ap()
    bmk_in = nc.dram_tensor("bmk_in", [1, KH], bf16, kind="ExternalInput").ap()
    # M2^T = [M; M]^T  [256, 128]  (duplicated so G lands in both halves)
    mt_in = nc.dram_tensor("mt_in", [C, 2 * MID], bf16, kind="ExternalInput").ap()
    vwt_in = nc.dram_tensor("vwt_in", [C, C], bf16, kind="ExternalInput").ap()
    kw1f_in = nc.dram_tensor("kw1f_in", [1, 2 * MID], bf16, kind="ExternalInput").ap()
    beta_in = nc.dram_tensor("beta_in", [2 * MID, 1], f32, kind="ExternalInput").ap()
    out_d = nc.dram_tensor("outp", [C, HW], f32, kind="ExternalOutput").ap()

    # j-chunking of the 4096 axis: two 1536 chunks + one 1024 chunk.
    # PSUM: "big" slots [128,1536] (3 banks) x2 bufs + "small" (1 bank) x2 = 8.
    CHUNKS = [(0, 1536), (1536, 1536), (3072, 1024)]
    C_CHUNKS = CHUNKS

    with tile.TileContext(nc) as tc:
        with (
            tc.tile_pool(name="sb", bufs=1) as sb,
            tc.tile_pool(name="ost", bufs=2) as osp,
            tc.tile_pool(name="ps", bufs=2, space="PSUM") as ps,
        ):
            # ---- weights / inputs; u on sync queue, the rest on gpsimd ----
            mt = sb.tile([128, 2 * MID], bf16, tag="mt", name="mt")
            nc.gpsimd.dma_start(mt[0:128, :], mt_in[0:128, :])
            mt1 = sb.tile([128, 2 * MID], bf16, tag="mt1", name="mt1")
            nc.gpsimd.dma_start(mt1[0:128, :], mt_in[128:256, :])
            kw1 = sb.tile([1, 2 * MID], bf16, tag="kw1", name="kw1")
            nc.gpsimd.dma_start(kw1[:], kw1f_in[:, :])
            betat = sb.tile([2 * MID, 1], f32, tag="betat", name="betat")
            nc.gpsimd.dma_start(betat[:], beta_in[:, :])
            bmk = sb.tile([1, KH], bf16, tag="bmk", name="bmk")
            nc.gpsimd.dma_start(bmk[:], bmk_in[:, :])
            u0 = sb.tile([128, HW], bf16, tag="u0", name="u0")
            u1 = sb.tile([128, HW], bf16, tag="u1", name="u1")
            for jo, w in CHUNKS:
                nc.sync.dma_start(u0[:, jo : jo + w], u_in[0:128, jo : jo + w])
                nc.sync.dma_start(u1[:, jo : jo + w], u_in[128:256, jo : jo + w])
            vwt0 = sb.tile([128, C], bf16, tag="vwt0", name="vwt0")
            nc.gpsimd.dma_start(vwt0[:], vwt_in[0:128, :])
            vwt1 = sb.tile([128, C], bf16, tag="vwt1", name="vwt1")
            nc.gpsimd.dma_start(vwt1[:], vwt_in[128:256, :])
            uk0 = sb.tile([128, KH], bf16, tag="uk0", name="uk0")
            nc.gpsimd.dma_start(uk0[:], uk_in[0:128, :])
            uk1 = sb.tile([128, KH], bf16, tag="uk1", name="uk1")
            nc.gpsimd.dma_start(uk1[:], uk_in[128:256, :])

            # ---- R2 = relu(kw1f2 outer bmk + beta2): both halves [128, 2048] ----
            R2 = sb.tile([128, KH], bf16, tag="R2", name="R2")
            for rc in range(KH // 512):
                pr = ps.tile([128, 512], f32, tag="small", bufs=2, name=f"pr{rc}")
                nc.tensor.matmul(
                    pr[:], kw1[:, :], bmk[:, rc * 512 : (rc + 1) * 512],
                    start=True, stop=True,
                )
                nc.scalar.activation(
                    R2[:, rc * 512 : (rc + 1) * 512], pr[:], AF.Relu,
                    bias=betat[:, 0:1],
                )

            # ---- G2 = M2 @ u (both halves) + first k-tile energy interleaved ----
            G2 = sb.tile([128, HW], bf16, tag="G2", name="G2")
            s_all = sb.tile([128, NK], f32, tag="s_all", name="s_all")
            rinv_all = sb.tile([128, NK], f32, tag="rinv", name="rinv_all")
            sp_tail = {}
            for kt in range(NK - 2, NK):
                sp_tail[kt] = sb.tile([128, 4], f32, tag=f"sp{kt}", name=f"sp{kt}")
            u_pairs = []
            for pair in range(NP):
                Up = sb.tile([128, 2 * HW], fp8, tag=f"Up{pair}", name=f"Up{pair}")
                u_pairs.append(Up)
            vtb = []
            for kt in range(NK):
                v = sb.tile([128, C], bf16, tag=f"vtb{kt}", name=f"vtb{kt}")
                vtb.append(v)
            vtsp = []
            for pair in range(NP):
                vp = sb.tile([128, 2 * C], fp8, tag=f"vtsp{pair}", name=f"vtsp{pair}")
                vtsp.append(vp)

            N_ACC = 2  # last k-tiles whose row-sum rides the ACT accumulator

            def energy_chunk(kt, ci):
                """Energy matmuls + exp for one (k-tile, j-chunk)."""
                pair, half = kt // 2, kt % 2
                Up = u_pairs[pair]
                accum_tail = kt >= NK - N_ACC
                jo, w = CHUNKS[ci]
                pe = ps.tile([128, 1536], f32, tag="big", name=f"pe{kt}_{jo}")
                nq = w // 512
                for q in range(0, nq, 2):
                    # concurrent partition-half duo (K=64 row groups)
                    js0 = jo + q * 512
                    nc.tensor.matmul(
                        pe[:, q * 512 : (q + 1) * 512],
                        R2[0:64, kt * 128 : (kt + 1) * 128],
                        G2[0:64, js0 : js0 + 512],
                        start=True, stop=True,
                    )
                    if q + 1 < nq:
                        js1 = jo + (q + 1) * 512
                        nc.tensor.matmul(
                            pe[:, (q + 1) * 512 : (q + 2) * 512],
                            R2[64:128, kt * 128 : (kt + 1) * 128],
                            G2[64:128, js1 : js1 + 512],
                            start=True, stop=True,
                        )
                nc.scalar.activation(
                    Up[:, half * HW + jo : half * HW + jo + w],
                    pe[:, 0:w], AF.Exp,
                    accum_out=(sp_tail[kt][:, ci : ci + 1] if accum_tail else None),
                )
                if accum_tail and ci == len(CHUNKS) - 1:
                    nc.vector.reduce_sum(
                        s_all[:, kt : kt + 1], sp_tail[kt][:, 0:3], axis=AX.X
                    )

            def ktile_epilogue(kt):
                """Row-sum (if not ACT-accumulated) + Vt matmul pair + scales."""
                pair, half = kt // 2, kt % 2
                if kt < NK - N_ACC:
                    nc.vector.reduce_sum(
                        s_all[:, kt : kt + 1],
                        u_pairs[pair][:, half * HW : (half + 1) * HW], axis=AX.X,
                    )
                pv = ps.tile([128, C], f32, tag="small", bufs=2, name=f"pv{kt}")
                ko = kt * 128
                nc.tensor.matmul(
                    pv[:], uk0[:, ko : ko + 128], vwt0[:, :], start=True, stop=False
                )
                nc.tensor.matmul(
                    pv[:], uk1[:, ko : ko + 128], vwt1[:, :], start=False, stop=True
                )
                nc.vector.tensor_copy(vtb[kt][:], pv[:])
                if half == 1:
                    nc.vector.reciprocal(
                        rinv_all[:, kt - 1 : kt + 1], s_all[:, kt - 1 : kt + 1]
                    )
                    for h2 in (0, 1):
                        nc.gpsimd.tensor_scalar(
                            vtsp[pair][:, h2 * C : (h2 + 1) * C],
                            vtb[kt - 1 + h2][:],
                            rinv_all[:, kt - 1 + h2 : kt + h2], VSCALE,
                            op0=ALU.mult, op1=ALU.mult,
                        )

            # G chunk production interleaved chunk-major with k-tiles 0 and 1,
            # so ACT has exp work while later G chunks are still being built
            for ci, (jo, w) in enumerate(CHUNKS):
                pg = ps.tile([128, 1536], f32, tag="big", name=f"pg{jo}")
                for q in range(w // 512):
                    sl = slice(q * 512, (q + 1) * 512)
                    js = jo + q * 512
                    nc.tensor.matmul(
                        pg[:, sl], mt[:, :], u0[:, js : js + 512],
                        start=True, stop=False,
                    )
                    nc.tensor.matmul(
                        pg[:, sl], mt1[:, :], u1[:, js : js + 512],
                        start=False, stop=True,
                    )
                nc.vector.tensor_copy(G2[:, jo : jo + w], pg[:, 0:w])
                energy_chunk(0, ci)
                energy_chunk(1, ci)
            ktile_epilogue(0)
            ktile_epilogue(1)
            for kt in range(2, NK):
                for ci in range(len(CHUNKS)):
                    energy_chunk(kt, ci)
                ktile_epilogue(kt)

            # ---- P = Vt^T @ U  (fp8 DoubleRow: 2 keys/cell) -> DRAM ----
            DR = mybir.MatmulPerfMode.DoubleRow
            for ct in range(2):
                for jg, (jo, w) in enumerate(C_CHUNKS):
                    po = ps.tile([128, 1536], f32, tag="big", name=f"po{ct}_{jg}")
                    for pair in range(NP):
                        lhsT = vtsp[pair].rearrange("p (i c) -> p i c", i=2)[
                            :, :, ct * 128 : (ct + 1) * 128
                        ]
                        for q in range(w // 512):
                            sl = slice(q * 512, (q + 1) * 512)
                            js = jo + q * 512
                            rhs = u_pairs[pair].rearrange("p (i j) -> p i j", i=2)[
                                :, :, js : js + 512
                            ]
                            nc.tensor.matmul(
                                po[:, sl], lhsT, rhs,
                                start=(pair == 0), stop=(pair == NP - 1),
                                perf_mode=DR,
                            )
                    ost = osp.tile([128, 1536], f32, tag="ost", name=f"ost{ct}_{jg}")
                    if ct == 1 and jg == len(C_CHUNKS) - 1:
                        # final group: split copy/DMA halves to shorten the tail
                        h = w // 2
                        nc.scalar.copy(ost[:, 0:h], po[:, 0:h])
                        nc.sync.dma_start(
                            out_d[ct * 128 : (ct + 1) * 128, jo : jo + h],
                            ost[:, 0:h],
                        )
                        nc.scalar.copy(ost[:, h:w], po[:, h:w])
                        nc.scalar.dma_start(
                            out_d[ct * 128 : (ct + 1) * 128, jo + h : jo + w],
                            ost[:, h:w],
                        )
                    else:
                        nc.scalar.copy(ost[:, 0:w], po[:, 0:w])
                        nc.sync.dma_start(
                            out_d[ct * 128 : (ct + 1) * 128, jo : jo + w],
                            ost[:, 0:w],
                        )

    nc.compile()
    return nc


def _get_built():
    global _BUILT
    if _BUILT is None:
        _BUILT = _build()
    return _BUILT


def _kperm():
    """Pairwise interleave within 256-key blocks: new index kt*128+q maps to
    old key  (kt//2)*256 + 2q + (kt%2)."""
    perm = np.empty(KH, np.int64)
    for pair in range(NP):
        base = pair * 256
        perm[pair * 256 : pair * 256 + 128] = base + np.arange(0, 256, 2)
        perm[pair * 256 + 128 : pair * 256 + 256] = base + np.arange(1, 256, 2)
    return perm


def _host_prep(boundary_map, uncertainty_map, key_w1, bn_scale, bn_bias,
               bn_mean, bn_var, key_w2, query_w, value_w):
    import ml_dtypes

    bf16 = ml_dtypes.bfloat16
    b, c, h, w = uncertainty_map.shape
    H0 = boundary_map.shape[2]
    idx = (np.arange(h) * H0) // h
    bm = boundary_map[:, 0][:, idx][:, :, idx].reshape(b, h * w).astype(np.float32)

    inv = bn_scale / np.sqrt(bn_var + 1e-5)
    beta = (bn_bias - bn_mean * inv).astype(np.float32)
    kw1f = (key_w1[:, 0] * inv).astype(np.float32)
    m_t = np.ascontiguousarray((key_w2.T @ query_w).T).astype(np.float32)  # [256, 64]
    # duplicate across partition halves for the energy duo-packing
    kw1f2 = np.concatenate([kw1f, kw1f]).reshape(1, 2 * MID).astype(bf16)
    beta2 = np.concatenate([beta, beta]).reshape(2 * MID, 1).astype(np.float32)
    m_t2 = np.concatenate([m_t, m_t], axis=1).astype(bf16)                 # [256, 128]
    vw_t = np.ascontiguousarray(value_w.T).astype(bf16)                    # [256, 256]
    perm = _kperm()

    in_maps = []
    for core in range(8):
        bi, kh = core // 2, core % 2
        u = np.ascontiguousarray(uncertainty_map[bi].reshape(c, h * w)).astype(bf16)
        uk = u[:, kh * KH : (kh + 1) * KH][:, perm]
        bmk = bm[bi, kh * KH : (kh + 1) * KH][perm]
        in_maps.append({
            "u_in": u,
            "uk_in": np.ascontiguousarray(uk),
            "bmk_in": np.ascontiguousarray(bmk).reshape(1, KH).astype(bf16),
            "mt_in": m_t2,
            "vwt_in": vw_t,
            "kw1f_in": kw1f2,
            "beta_in": beta2,
        })
    return in_maps


def kernel(boundary_map, uncertainty_map, key_w1, bn_scale, bn_bias,
           bn_mean, bn_var, key_w2, query_w, value_w, gamma):
    global LAST_RESULTS
    from concourse.bass_utils import run_bass_kernel_spmd

    nc = _get_built()
    in_maps = _host_prep(
        np.asarray(boundary_map), np.asarray(uncertainty_map), np.asarray(key_w1),
        np.asarray(bn_scale), np.asarray(bn_bias), np.asarray(bn_mean),
        np.asarray(bn_var), np.asarray(key_w2), np.asarray(query_w),
        np.asarray(value_w),
    )
    kwargs = {}
    if TRACE:
        kwargs["trace"] = True
        if TRACE_CORES is not None:
            kwargs["trace_cores"] = TRACE_CORES
    res = run_bass_kernel_spmd(nc, in_maps, core_ids=list(range(8)), **kwargs)
    LAST_RESULTS = res

    b, c, h, w = uncertainty_map.shape
    g = np.float32(np.asarray(gamma).reshape(-1)[0] / VSCALE)
    out = np.empty((b, c, h * w), np.float32)
    um = np.asarray(uncertainty_map)
    for bi in range(b):
        P = res.results[2 * bi]["outp"] + res.results[2 * bi + 1]["outp"]
        out[bi] = g * P + um[bi].reshape(c, h * w)
    return out.reshape(b, c, h, w)


# revision 26
# speedup vs baseline: 1.0053x; 1.0053x over previous
"""BoundaryAttentionModule Trainium2 kernel.

Shapes (hardcoded): b=4, c=256, h=w=64 (HW=4096), boundary 128x128,
mid=64, out_ch=256. 8 cores: core = (batch bi = core//2, key-half kh = core%2).

Math (exact reassociation of the reference):
  bm   = nearest-downsampled boundary map        [b, 4096]
  R    = relu(kw1f outer bm_khalf + beta)        [64, 2048]   (kw1f = key_w1*bn_inv)
  G    = (key_w2^T @ query_w) @ u                [64, 4096]
  E^T  = R^T @ G                                 [2048_k, 4096_j]  (logits tiny, no max sub)
  U    = exp(E^T), s[k] = sum_j U[k, j]
  Vt   = (u^T @ value_w^T)[k_half] / s * 8192    [2048, 256]
  P    = Vt^T @ U                                [256, 4096]  per-core partial (x8192)
host: out[bi] = (gamma/8192) * (P[2bi] + P[2bi+1]) + u[bi]

Energy matmuls in bf16 with K=64 contraction packed as concurrent
partition-half duos (R and G are host/device-duplicated into both
partition halves, so two j-slices of one k-tile run in the PE array
simultaneously).  The output matmul runs in fp8e4 DoubleRow (2 keys per
cell); the key axis is host-permuted pairwise so PSUM partitions
interleave — the permutation only reorders the contracted axis.
"""

import numpy as np

B, C, HW = 4, 256, 4096
KH = HW // 2          # 2048 keys per core
NK = KH // 128        # 16 k tiles
NP = NK // 2          # 8 k-tile pairs
MID = 64
VSCALE = 8192.0       # fp8 scaling of Vt (power of two; host divides gamma)

TRACE = False
TRACE_CORES = None
LAST_RESULTS = None

_BUILT = None


def _build():
    import concourse.bass as bass
    import concourse.tile as tile
    from concourse import bacc, mybir

    f32 = mybir.dt.float32
    bf16 = mybir.dt.bfloat16
    fp8 = mybir.dt.float8e4
    AF = mybir.ActivationFunctionType
    AX = mybir.AxisListType
    ALU = mybir.AluOpType

    nc = bacc.Bacc(
        "TRN2",
        target_bir_lowering=False,
        debug=False,
        enable_asserts=False,
        num_devices=8,
    )

    u_in = nc.dram_tensor("u_in", [C, HW], bf16, kind="ExternalInput").ap()
    uk_in = nc.dram_tensor("uk_in", [C, KH], bf16, kind="ExternalInput").ap()
    bmk_in = nc.dram_tensor("bmk_in", [1, KH], bf16, kind="ExternalInput").ap()
    # M2^T = [M; M]^T  [256, 128]  (duplicated so G lands in both halves)
    mt_in = nc.dram_tensor("mt_in", [C, 2 * MID], bf16, kind="ExternalInput").ap()
    vwt_in = nc.dram_tensor("vwt_in", [C, C], bf16, kind="ExternalInput").ap()
    kw1f_in = nc.dram_tensor("kw1f_in", [1, 2 * MID], bf16, kind="ExternalInput").ap()
    beta_in = nc.dram_tensor("beta_in", [2 * MID, 1], f32, kind="ExternalInput").ap()
    out_d = nc.dram_tensor("outp", [C, HW], f32, kind="ExternalOutput").ap()

    # j-chunking of the 4096 axis: two 1536 chunks + one 1024 chunk.
    # PSUM: "big" slots [128,1536] (3 banks) x2 bufs + "small" (1 bank) x2 = 8.
    CHUNKS = [(0, 1536), (1536, 1536), (3072, 1024)]
    C_CHUNKS = CHUNKS

    with tile.TileContext(nc) as tc:
        with (
            tc.tile_pool(name="sb", bufs=1) as sb,
            tc.tile_pool(name="ost", bufs=2) as osp,
            tc.tile_pool(name="ps", bufs=2, space="PSUM") as ps,
        ):
            # ---- weights / inputs; u on sync queue, the rest on gpsimd ----
            mt = sb.tile([128, 2 * MID], bf16, tag="mt", name="mt")
            nc.gpsimd.dma_start(mt[0:128, :], mt_in[0:128, :])
            mt1 = sb.tile([128, 2 * MID], bf16, tag="mt1", name="mt1")
            nc.gpsimd.dma_start(mt1[0:128, :], mt_in[128:256, :])
            kw1 = sb.tile([1, 2 * MID], bf16, tag="kw1", name="kw1")
            nc.gpsimd.dma_start(kw1[:], kw1f_in[:, :])
            betat = sb.tile([2 * MID, 1], f32, tag="betat", name="betat")
            nc.gpsimd.dma_start(betat[:], beta_in[:, :])
            bmk = sb.tile([1, KH], bf16, tag="bmk", name="bmk")
            nc.gpsimd.dma_start(bmk[:], bmk_in[:, :])
            u0 = sb.tile([128, HW], bf16, tag="u0", name="u0")
            u1 = sb.tile([128, HW], bf16, tag="u1", name="u1")
            for jo, w in CHUNKS:
                nc.sync.dma_start(u0[:, jo : jo + w], u_in[0:128, jo : jo + w])
                nc.sync.dma_start(u1[:, jo : jo + w], u_in[128:256, jo : jo + w])
            vwt0 = sb.tile([128, C], bf16, tag="vwt0", name="vwt0")
            nc.gpsimd.dma_start(vwt0[:], vwt_in[0:128, :])
            vwt1 = sb.tile([128, C], bf16, tag="vwt1", name="vwt1")
            nc.gpsimd.dma_start(vwt1[:], vwt_in[128:256, :])
            uk0 = sb.tile([128, KH], bf16, tag="uk0", name="uk0")
            nc.gpsimd.dma_start(uk0[:], uk_in[0:128, :])
            uk1 = sb.tile([128, KH], bf16, tag="uk1", name="uk1")
            nc.gpsimd.dma_start(uk1[:], uk_in[128:256, :])

            # ---- R2 = relu(kw1f2 outer bmk + beta2): both halves [128, 2048] ----
            R2 = sb.tile([128, KH], bf16, tag="R2", name="R2")
            for rc in range(KH // 512):
                pr = ps.tile([128, 512], f32, tag="small", bufs=2, name=f"pr{rc}")
                nc.tensor.matmul(
                    pr[:], kw1[:, :], bmk[:, rc * 512 : (rc + 1) * 512],
                    start=True, stop=True,
                )
                nc.scalar.activation(
                    R2[:, rc * 512 : (rc + 1) * 512], pr[:], AF.Relu,
                    bias=betat[:, 0:1],
                )

            # ---- G2 = M2 @ u (both halves) + first k-tile energy interleaved ----
            G2 = sb.tile([128, HW], bf16, tag="G2", name="G2")
            s_all = sb.tile([128, NK], f32, tag="s_all", name="s_all")
            rinv_all = sb.tile([128, NK], f32, tag="rinv", name="rinv_all")
            sp_tail = {}
            for kt in range(NK - 3, NK):
                sp_tail[kt] = sb.tile([128, 4], f32, tag=f"sp{kt}", name=f"sp{kt}")
            u_pairs = []
            for pair in range(NP):
                Up = sb.tile([128, 2 * HW], fp8, tag=f"Up{pair}", name=f"Up{pair}")
                u_pairs.append(Up)
            vtb = []
            for kt in range(NK):
                v = sb.tile([128, C], bf16, tag=f"vtb{kt}", name=f"vtb{kt}")
                vtb.append(v)
            vtsp = []
            for pair in range(NP):
                vp = sb.tile([128, 2 * C], fp8, tag=f"vtsp{pair}", name=f"vtsp{pair}")
                vtsp.append(vp)

            N_ACC = 3  # last k-tiles whose row-sum rides the ACT accumulator

            def energy_chunk(kt, ci):
                """Energy matmuls + exp for one (k-tile, j-chunk)."""
                pair, half = kt // 2, kt % 2
                Up = u_pairs[pair]
                accum_tail = kt >= NK - N_ACC
                jo, w = CHUNKS[ci]
                pe = ps.tile([128, 1536], f32, tag="big", name=f"pe{kt}_{jo}")
                nq = w // 512
                for q in range(0, nq, 2):
                    # concurrent partition-half duo (K=64 row groups)
                    js0 = jo + q * 512
                    nc.tensor.matmul(
                        pe[:, q * 512 : (q + 1) * 512],
                        R2[0:64, kt * 128 : (kt + 1) * 128],
                        G2[0:64, js0 : js0 + 512],
                        start=True, stop=True,
                    )
                    if q + 1 < nq:
                        js1 = jo + (q + 1) * 512
                        nc.tensor.matmul(
                            pe[:, (q + 1) * 512 : (q + 2) * 512],
                            R2[64:128, kt * 128 : (kt + 1) * 128],
                            G2[64:128, js1 : js1 + 512],
                            start=True, stop=True,
                        )
                nc.scalar.activation(
                    Up[:, half * HW + jo : half * HW + jo + w],
                    pe[:, 0:w], AF.Exp,
                    accum_out=(sp_tail[kt][:, ci : ci + 1] if accum_tail else None),
                )
                if accum_tail and ci == len(CHUNKS) - 1:
                    nc.vector.reduce_sum(
                        s_all[:, kt : kt + 1], sp_tail[kt][:, 0:3], axis=AX.X
                    )

            def ktile_epilogue(kt):
                """Row-sum (if not ACT-accumulated) + Vt matmul pair + scales."""
                pair, half = kt // 2, kt % 2
                if kt < NK - N_ACC:
                    nc.vector.reduce_sum(
                        s_all[:, kt : kt + 1],
                        u_pairs[pair][:, half * HW : (half + 1) * HW], axis=AX.X,
                    )
                pv = ps.tile([128, C], f32, tag="small", bufs=2, name=f"pv{kt}")
                ko = kt * 128
                nc.tensor.matmul(
                    pv[:], uk0[:, ko : ko + 128], vwt0[:, :], start=True, stop=False
                )
                nc.tensor.matmul(
                    pv[:], uk1[:, ko : ko + 128], vwt1[:, :], start=False, stop=True
                )
                nc.vector.tensor_copy(vtb[kt][:], pv[:])
                if half == 1:
                    nc.vector.reciprocal(
                        rinv_all[:, kt - 1 : kt + 1], s_all[:, kt - 1 : kt + 1]
                    )
                    for h2 in (0, 1):
                        nc.gpsimd.tensor_scalar(
                            vtsp[pair][:, h2 * C : (h2 + 1) * C],
                            vtb[kt - 1 + h2][:],
                            rinv_all[:, kt - 1 + h2 : kt + h2], VSCALE,
                            op0=ALU.mult, op1=ALU.mult,
                        )

            # G chunk production interleaved chunk-major with k-tiles 0 and 1,
            # so ACT has exp work while later G chunks are still being built
            for ci, (jo, w) in enumerate(CHUNKS):
                pg = ps.tile([128, 1536], f32, tag="big", name=f"pg{jo}")
                for q in range(w // 512):
                    sl = slice(q * 512, (q + 1) * 512)
                    js = jo + q * 512
                    nc.tensor.matmul(
                        pg[:, sl], mt[:, :], u0[:, js : js + 512],
                        start=True, stop=False,
                    )
                    nc.tensor.matmul(
                        pg[:, sl], mt1[:, :], u1[:, js : js + 512],
                        start=False, stop=True,
                    )
                nc.vector.tensor_copy(G2[:, jo : jo + w], pg[:, 0:w])
                energy_chunk(0, ci)
                energy_chunk(1, ci)
            ktile_epilogue(0)
            ktile_epilogue(1)
            for kt in range(2, NK):
                for ci in range(len(CHUNKS)):
                    energy_chunk(kt, ci)
                ktile_epilogue(kt)

            # ---- P = Vt^T @ U  (fp8 DoubleRow: 2 keys/cell) -> DRAM ----
            DR = mybir.MatmulPerfMode.DoubleRow
            for ct in range(2):
                for jg, (jo, w) in enumerate(C_CHUNKS):
                    po = ps.tile([128, 1536], f32, tag="big", name=f"po{ct}_{jg}")
                    for pair in range(NP):
                        lhsT = vtsp[pair].rearrange("p (i c) -> p i c", i=2)[
                            :, :, ct * 128 : (ct + 1) * 128
                        ]
                        for q in range(w // 512):
                            sl = slice(q * 512, (q + 1) * 512)
                            js = jo + q * 512
                            rhs = u_pairs[pair].rearrange("p (i j) -> p i j", i=2)[
                                :, :, js : js + 512
                            ]
                            nc.tensor.matmul(
                                po[:, sl], lhsT, rhs,
                                start=(pair == 0), stop=(pair == NP - 1),
                                perf_mode=DR,
                            )
                    ost = osp.tile([128, 1536], f32, tag="ost", name=f"ost{ct}_{jg}")
                    if ct == 1 and jg == len(C_CHUNKS) - 1:
                        # final group: split copy/DMA halves to shorten the tail
                        h = w // 2
                        nc.scalar.copy(ost[:, 0:h], po[:, 0:h])
                        nc.sync.dma_start(
                            out_d[ct * 128 : (ct + 1) * 128, jo : jo + h],
                            ost[:, 0:h],
                        )
                        nc.scalar.copy(ost[:, h:w], po[:, h:w])
                        nc.scalar.dma_start(
                            out_d[ct * 128 : (ct + 1) * 128, jo + h : jo + w],
                            ost[:, h:w],
                        )
                    else:
                        nc.scalar.copy(ost[:, 0:w], po[:, 0:w])
                        nc.sync.dma_start(
                            out_d[ct * 128 : (ct + 1) * 128, jo : jo + w],
                            ost[:, 0:w],
                        )

    nc.compile()
    return nc


def _get_built():
    global _BUILT
    if _BUILT is None:
        _BUILT = _build()
    return _BUILT


def _kperm():
    """Pairwise interleave within 256-key blocks: new index kt*128+q maps to
    old key  (kt//2)*256 + 2q + (kt%2)."""
    perm = np.empty(KH, np.int64)
    for pair in range(NP):
        base = pair * 256
        perm[pair * 256 : pair * 256 + 128] = base + np.arange(0, 256, 2)
        perm[pair * 256 + 128 : pair * 256 + 256] = base + np.arange(1, 256, 2)
    return perm


def _host_prep(boundary_map, uncertainty_map, key_w1, bn_scale, bn_bias,
               bn_mean, bn_var, key_w2, query_w, value_w):
    import ml_dtypes

    bf16 = ml_dtypes.bfloat16
    b, c, h, w = uncertainty_map.shape
    H0 = boundary_map.shape[2]
    idx = (np.arange(h) * H0) // h
    bm = boundary_map[:, 0][:, idx][:, :, idx].reshape(b, h * w).astype(np.float32)

    inv = bn_scale / np.sqrt(bn_var + 1e-5)
    beta = (bn_bias - bn_mean * inv).astype(np.float32)
    kw1f = (key_w1[:, 0] * inv).astype(np.float32)
    m_t = np.ascontiguousarray((key_w2.T @ query_w).T).astype(np.float32)  # [256, 64]
    # duplicate across partition halves for the energy duo-packing
    kw1f2 = np.concatenate([kw1f, kw1f]).reshape(1, 2 * MID).astype(bf16)
    beta2 = np.concatenate([beta, beta]).reshape(2 * MID, 1).astype(np.float32)
    m_t2 = np.concatenate([m_t, m_t], axis=1).astype(bf16)                 # [256, 128]
    vw_t = np.ascontiguousarray(value_w.T).astype(bf16)                    # [256, 256]
    perm = _kperm()

    in_maps = []
    for core in range(8):
        bi, kh = core // 2, core % 2
        u = np.ascontiguousarray(uncertainty_map[bi].reshape(c, h * w)).astype(bf16)
        uk = u[:, kh * KH : (kh + 1) * KH][:, perm]
        bmk = bm[bi, kh * KH : (kh + 1) * KH][perm]
        in_maps.append({
            "u_in": u,
            "uk_in": np.ascontiguousarray(uk),
            "bmk_in": np.ascontiguousarray(bmk).reshape(1, KH).astype(bf16),
            "mt_in": m_t2,
            "vwt_in": vw_t,
            "kw1f_in": kw1f2,
            "beta_in": beta2,
        })
    return in_maps


def kernel(boundary_map, uncertainty_map, key_w1, bn_scale, bn_bias,
           bn_mean, bn_var, key_w2, query_w, value_w, gamma):
    global LAST_RESULTS
    from concourse.bass_utils import run_bass_kernel_spmd

    nc = _get_built()
    in_maps = _host_prep(
        np.asarray(boundary_map), np.asarray(uncertainty_map), np.asarray(key_w1),
        np.asarray(bn_scale), np.asarray(bn_bias), np.asarray(bn_mean),
        np.asarray(bn_var), np.asarray(key_w2), np.asarray(query_w),
        np.asarray(value_w),
    )
    kwargs = {}
    if TRACE:
        kwargs["trace"] = True
        if TRACE_CORES is not None:
            kwargs["trace_cores"] = TRACE_CORES
    res = run_bass_kernel_spmd(nc, in_maps, core_ids=list(range(8)), **kwargs)
    LAST_RESULTS = res

    b, c, h, w = uncertainty_map.shape
    g = np.float32(np.asarray(gamma).reshape(-1)[0] / VSCALE)
    out = np.empty((b, c, h * w), np.float32)
    um = np.asarray(uncertainty_map)
    for bi in range(b):
        P = res.results[2 * bi]["outp"] + res.results[2 * bi + 1]["outp"]
        out[bi] = g * P + um[bi].reshape(c, h * w)
    return out.reshape(b, c, h, w)
